# revision 30
# baseline (speedup 1.0000x reference)
"""Trainium2 Bass kernel for nn_Attention_49168785605257.

Causal multi-head self-attention: B=2, N=4096, DIM=512, H=8, DH=64.
Reference applies dim_head**-0.5 scaling TWICE; folded here into the exp's
scale parameter (1/64) together with the 16x q/k weight pre-scales
(1/16384 net).

Sharding: one head per NeuronCore (8 cores). Each core computes its head's
UN-normalized partial output projection o_h = (sum_j w_ij v_j) @ Wo_h and its
softmax denominator row; the host divides each core's partial by its den row
(row scaling commutes with the Wo projection), sums the 8 results and adds
the bias.  Dropping on-device normalization removes the per-chunk
bounce->reciprocal->partition-broadcast serial chain from the critical path.

Device-side formulation (per core):
  - All tensors carried transposed ([feature, token]); host pre-transposes x.
  - q and k projections fused into ONE fp8e4 DoubleRow matmul chain per
    batch-chunk (stationary [Wq|Wk] is 128 wide, two 256-deep k-tile pairs,
    0.5 cyc/row); the fp32 PSUM is cast to bf16 and DMA'd into qT/kT (DMA
    shifts partitions; engines cannot).  DoubleRow only works on full-PE
    (128,128) tiles at position (0,0), so S^T stays bf16 on quadrant tiles.
  - Flash-attention in S^T orientation: exp on ScalarE with scale=1/16384
    (PSUM->SBUF, [128,1024] groups covering both batches); causal masking by
    multiplying the exp output of the diagonal j-blocks with 0/1 masks; A@V
    accumulated in PSUM with v augmented by a ones-column so row 64 collects
    the denominators (FP8_AV switches the sub-diagonal blocks to fp8e4
    DoubleRow over paired j-blocks).
  - DMA routing: bulk x loads merged to 2 DMAs/chunk, issued on sync (SP)
    for the startup chunks and on gpsimd (Pool SWDGE) afterwards — a HWDGE
    dma_start occupies the issuing engine's sequencer for ~650ns, so they
    must stay off the Scalar queue (exp dispatch) in steady state.  Outputs
    merged to one DMA/chunk on sync; qT/kT repack on sync.
"""

import os
import sys
from contextlib import ExitStack

import numpy as np

for _p in ("/opt/trn_rl_repo", "/root/.axon_site/_ro/trn_rl_repo"):
    if _p not in sys.path and os.path.isdir(_p):
        sys.path.append(_p)

import ml_dtypes  # noqa: E402

B, N, DIM, H, DH = 2, 4096, 512, 8, 64
N_CORES = 8
CH = 512            # i-chunk width (tokens)
JB = 128            # j-block width (tokens)
EXP_SCALE = 1.0 / 16384.0   # 1/64 (double dim_head**-0.5) * 1/256 (16x Wq,Wk)
FP8_AV = True       # fp8e4 DoubleRow A@V over paired sub-diagonal j-blocks
DR_PROJ = True      # fp8e4 DoubleRow fused q+k projection (else bf16 baseline)

BF16 = "bfloat16"
F32 = "float32"


def _pin_act_tables():
    """Make Exp resolve only to one table set so the kernel never swaps ACT
    table sets mid-run. Best-effort."""
    try:
        import concourse.bacc as bacc
        import concourse.hw_specs as hw_specs
        import concourse.mybir as mybir
        orig = hw_specs.get_activation_tables

        def patched(module_arch):
            try:
                tabs = dict(orig(module_arch))
                both = {mybir.ActivationFunctionType.Exp, mybir.ActivationFunctionType.Ln}
                target = None
                for name, funcs in tabs.items():
                    if both <= funcs:
                        target = name
                        break
                if target is None:
                    return tabs
                out = {}
                for name, funcs in tabs.items():
                    out[name] = set(funcs) if name == target else set(funcs) - both
                return out
            except Exception:
                return orig(module_arch)

        bacc.get_activation_tables = patched
    except Exception:
        pass


def build_attention_kernel(nc, NB: int):
    """Emit the per-core program. NB = tokens per batch (4096 full size)."""
    import concourse.mybir as mybir
    import concourse.tile as tile
    _pin_act_tables()

    bf16 = mybir.dt.bfloat16
    f32 = mybir.dt.float32
    fp8 = mybir.dt.float8e4
    mult = mybir.AluOpType.mult
    Exp = mybir.ActivationFunctionType.Exp
    DR = mybir.MatmulPerfMode.DoubleRow

    NCH = NB // CH          # i-chunks per batch
    JTB = NB // JB          # j-blocks per batch

    xT_d = nc.dram_tensor("xT", [DIM, 2 * NB], bf16, kind="ExternalInput").ap()
    if DR_PROJ:
        x8_d = nc.dram_tensor("x8", [128, 8 * NB], fp8, kind="ExternalInput").ap()
        wqk8_d = nc.dram_tensor("wqk8", [128, 512], fp8, kind="ExternalInput").ap()
    else:
        wqb_d = nc.dram_tensor("wqb", [128, 4 * DH], bf16, kind="ExternalInput").ap()
        wkb_d = nc.dram_tensor("wkb", [128, 4 * DH], bf16, kind="ExternalInput").ap()
    wv_d = nc.dram_tensor("wv", [128, 4 * DH], bf16, kind="ExternalInput").ap()
    wo_d = nc.dram_tensor("wo", [DH, DIM], bf16, kind="ExternalInput").ap()
    mask_d = nc.dram_tensor("masks", [128, 256], bf16, kind="ExternalInput").ap()
    idup_d = nc.dram_tensor("identup", [128, DH], bf16, kind="ExternalInput").ap()
    oT_d = nc.dram_tensor("oT", [DIM, 2 * NB], bf16, kind="ExternalOutput").ap()
    den_d = nc.dram_tensor("den", [1, 2 * NB], f32, kind="ExternalOutput").ap()

    xT_r = xT_d.rearrange("(g p) n -> p g n", g=4)
    oT_r = oT_d.rearrange("(g p) n -> p g n", g=4)

    with tile.TileContext(nc) as tc, ExitStack() as ctx:
        const = ctx.enter_context(tc.tile_pool(name="const", bufs=1))
        xpool = ctx.enter_context(tc.tile_pool(name="xp", bufs=3))
        x8pool = ctx.enter_context(tc.tile_pool(name="x8p", bufs=3))
        big = ctx.enter_context(tc.tile_pool(name="big", bufs=1))
        ptp = ctx.enter_context(tc.tile_pool(name="ptp", bufs=8))
        q8p = ctx.enter_context(tc.tile_pool(name="q8p", bufs=3))
        rp = ctx.enter_context(tc.tile_pool(name="rp", bufs=3))
        op_sb_pool = ctx.enter_context(tc.tile_pool(name="osb", bufs=2))
        ps_pool = ctx.enter_context(tc.tile_pool(name="ps", bufs=2, space="PSUM"))
        av_pool = ctx.enter_context(tc.tile_pool(name="av", bufs=1, space="PSUM"))
        pv_pool = ctx.enter_context(tc.tile_pool(name="pv", bufs=2, space="PSUM"))

        # ---- weights first so chunk-0 projections can start ASAP ----
        if DR_PROJ:
            wqk8_sb = const.tile([128, 512], fp8, tag="wqk8")
            nc.sync.dma_start(wqk8_sb[:], wqk8_d[:, :])
        else:
            wqb_sb = const.tile([128, 4 * DH], bf16, tag="wqb")
            wkb_sb = const.tile([128, 4 * DH], bf16, tag="wkb")
            nc.sync.dma_start(wqb_sb[:], wqb_d[:, :])
            nc.sync.dma_start(wkb_sb[:], wkb_d[:, :])
        wv_sb = const.tile([128, 4 * DH], bf16, tag="wv")
        nc.sync.dma_start(wv_sb[:], wv_d[:, :])
        wo_sb = const.tile([DH, DIM], bf16, tag="wo")
        mask_sb = const.tile([128, 256], bf16, tag="mask")
        idup_sb = const.tile([128, DH], bf16, tag="idup")

        # ---- persistent activations (partition halves: rows 0-63 batch0, 64-127 batch1) ----
        qT = big.tile([128, NB], bf16, tag="qT")
        kT = big.tile([128, NB], bf16, tag="kT")
        vT = big.tile([128, NB], bf16, tag="vT")
        vaug = [big.tile([128, 65 * JTB], bf16, tag=f"vaug{b}", name=f"vaug{b}")
                for b in range(2)]
        # fp8 twin of vaug padded to 128 cols/block: dual-fp8 LoadStationary
        # only accepts full (128,128) PE tiles. cols 0-63 v, 64 ones, 65+ zero.
        vaug8 = [big.tile([128, 128 * JTB], fp8, tag=f"vaug8{b}", name=f"vaug8{b}")
                 for b in range(2)] if FP8_AV else None

        def r3(t2, t=2):
            return t2.rearrange("p (t n) -> p t n", t=t)

        xts_pend = {}
        x8s_pend = {}

        def emit_x8(c):
            """fp8 x pairs for the q/k projections (on the critical path to
            the chunk's first S — issue early).  Startup chunks on the scalar
            queue (idle then); later ones on gpsimd SWDGE so neither the SP
            nor Scalar sequencer pays the ~650ns DMA dispatch when busy."""
            if not DR_PROJ:
                return
            eng = nc.scalar if c < 2 else nc.gpsimd
            x8r = x8_d.rearrange("p (pr t n) -> p pr t n", pr=2, t=2)
            x84 = x8pool.tile([128, 4096], fp8, tag="x8t", name=f"x8t{c}")
            eng.dma_start(x84[:].rearrange("p (pr t n) -> p pr t n", pr=2, t=2),
                          x8r[:, :, :, 1024 * c:1024 * (c + 1)])
            x8s_pend[c] = x84

        def emit_xtb(c):
            """bf16 x for the v projection — only needed by mid-chunk, so
            issued later to keep the startup DMA burst small."""
            eng = nc.scalar if c < 2 else nc.gpsimd
            xt4 = xpool.tile([128, 4096], bf16, tag="xt", name=f"xt{c}")
            eng.dma_start(xt4[:].rearrange("p (g n) -> p g n", g=4),
                          xT_r[:, :, 1024 * c:1024 * (c + 1)])
            xts_pend[c] = xt4

        def emit_xt(c):
            emit_x8(c)
            emit_xtb(c)

        def emit_chunk_prep(c, ws=(0, 1, 2, 3)):
            """ws: 0=q proj, 1=k proj, 2=v proj, 3=v transposes."""
            i0 = CH * c
            if c == 0 and 0 in ws:
                nc.sync.dma_start(idup_sb[:], idup_d[:, :])
            if DR_PROJ and (0 in ws or 1 in ws):
                x84 = x8s_pend[c]
                x8v = x84[:].rearrange("p (pr t n) -> p pr t n", pr=2, t=2)
                for b in (([0] if 0 in ws else []) + ([1] if 1 in ws else [])):
                    ps = pv_pool.tile([128, CH], f32, tag="pv", name=f"qk{b}")
                    for pr in range(2):
                        nc.tensor.matmul(
                            ps[:, :],
                            r3(wqk8_sb[:, 256 * pr:256 * (pr + 1)])[:, :, :],
                            x8v[:, pr, :, 512 * b:512 * (b + 1)],
                            start=(pr == 0), stop=(pr == 1), perf_mode=DR,
                            skip_group_check=True)
                    stg = q8p.tile([128, CH], bf16, tag="stg", name=f"stg{b}")
                    nc.vector.tensor_copy(stg[:], ps[:, :])
                    nc.sync.dma_start(qT[64 * b:64 * (b + 1), i0:i0 + CH], stg[0:64, :])
                    nc.sync.dma_start(kT[64 * b:64 * (b + 1), i0:i0 + CH], stg[64:128, :])
            elif 0 in ws or 1 in ws:
                xt4 = xts_pend[c]
                for w_sb, dst in (((wqb_sb, qT),) if 0 in ws else ()) + \
                                 (((wkb_sb, kT),) if 1 in ws else ()):
                    ps = pv_pool.tile([128, CH], f32, tag="pv", name="qkb")
                    for d in range(4):
                        nc.tensor.matmul(ps[0:64, :], w_sb[:, d * DH:(d + 1) * DH],
                                         xt4[:, 1024 * d:1024 * d + 512],
                                         start=(d == 0), stop=(d == 3), tile_position=(0, 0),
                                         skip_group_check=True)
                        nc.tensor.matmul(ps[64:128, :], w_sb[:, d * DH:(d + 1) * DH],
                                         xt4[:, 1024 * d + 512:1024 * (d + 1)],
                                         start=(d == 0), stop=(d == 3), tile_position=(0, 64),
                                         skip_group_check=True)
                    nc.vector.tensor_copy(dst[:, i0:i0 + CH], ps[:, :])
            if c == 0 and 2 in ws:
                nc.sync.dma_start(mask_sb[:], mask_d[:, :])
                nc.sync.dma_start(wo_sb[:], wo_d[:, :])
            if 2 in ws:
                # --- v projection (bf16) ---
                xt4 = xts_pend.pop(c)
                x8s_pend.pop(c, None)
                psv = pv_pool.tile([128, CH], f32, tag="pv", name="vproj")
                for d in range(4):
                    nc.tensor.matmul(psv[0:64, :], wv_sb[:, d * DH:(d + 1) * DH],
                                     xt4[:, 1024 * d:1024 * d + 512],
                                     start=(d == 0), stop=(d == 3), tile_position=(0, 0),
                                     skip_group_check=True)
                    nc.tensor.matmul(psv[64:128, :], wv_sb[:, d * DH:(d + 1) * DH],
                                     xt4[:, 1024 * d + 512:1024 * (d + 1)],
                                     start=(d == 0), stop=(d == 3), tile_position=(0, 64),
                                     skip_group_check=True)
                nc.vector.tensor_copy(vT[:, i0:i0 + CH], psv[:, :])
            if 3 not in ws:
                return
            # --- v transposes into vaug (ones-column augmented) ---
            for tt in range(4 * c, 4 * c + 4):
                pst0 = pv_pool.tile([128, 64], bf16, tag="pv", name="pst0")
                pst1 = pv_pool.tile([128, 64], bf16, tag="pv", name="pst1")
                nc.tensor.matmul(pst0[:], vT[0:64, JB * tt:JB * (tt + 1)], idup_sb[0:64, :],
                                 is_transpose=True, tile_position=(0, 0), skip_group_check=True)
                nc.tensor.matmul(pst1[:], vT[64:128, JB * tt:JB * (tt + 1)], idup_sb[64:128, :],
                                 is_transpose=True, tile_position=(64, 0), skip_group_check=True)
                nc.vector.tensor_copy(vaug[0][:, 65 * tt:65 * tt + 64], pst0[:])
                nc.vector.tensor_copy(vaug[1][:, 65 * tt:65 * tt + 64], pst1[:])
                if FP8_AV:
                    nc.vector.tensor_copy(vaug8[0][:, 128 * tt:128 * tt + 64], pst0[:])
                    nc.vector.tensor_copy(vaug8[1][:, 128 * tt:128 * tt + 64], pst1[:])

        def emit_epilogue_a(c, pso):
            """Evacuate A@V PSUM: un-normalized bf16 output for the Wo
            projection, fp32 denominator row streamed to the host."""
            outT_un = rp.tile([65, 1024], f32, tag="outT_un")
            nc.vector.tensor_copy(outT_un[:], pso[0:65, 0:1024])
            nc.sync.dma_start(den_d[0:1, 1024 * c:1024 * (c + 1)], outT_un[64:65, :])
            outTn = rp.tile([64, 1024], bf16, tag="outTn")
            nc.vector.tensor_copy(outTn[:], outT_un[0:64, :])
            return outTn

        o4_pend = {}

        def emit_epilogue_b(c, outTn, dblks=(0, 1, 2, 3)):
            """Deferred per-chunk tail: project; stream out once complete."""
            o4 = o4_pend.get(c)
            if o4 is None:
                o4 = op_sb_pool.tile([128, 4096], bf16, tag="o")
                o4_pend[c] = o4
            for dblk in dblks:
                for b in range(2):
                    opp = pv_pool.tile([128, 512], f32, tag="pv", name=f"opp{b}")
                    nc.tensor.matmul(opp[:], wo_sb[:, 128 * dblk:128 * (dblk + 1)],
                                     outTn[0:64, 512 * b:512 * b + 512],
                                     skip_group_check=True)
                    nc.vector.tensor_copy(
                        o4[:, 1024 * dblk + 512 * b:1024 * dblk + 512 * (b + 1)], opp[:])
                nc.gpsimd.dma_start(oT_r[:, dblk, 1024 * c:1024 * (c + 1)],
                                    o4[:, 1024 * dblk:1024 * (dblk + 1)])
            if dblks[-1] == 3:
                o4_pend.pop(c)

        def emit_s(c, jb, off):
            """S^T matmuls for block (c, jb) -> fresh PSUM tile."""
            i0 = CH * c
            pss = ps_pool.tile([128, 1024], f32, tag="s")
            nc.tensor.matmul(pss[:, off:512], kT[0:64, JB * jb:JB * (jb + 1)],
                             qT[0:64, i0 + off:i0 + CH],
                             start=True, stop=True, tile_position=(0, 0), skip_group_check=True)
            nc.tensor.matmul(pss[:, 512 + off:1024], kT[64:128, JB * jb:JB * (jb + 1)],
                             qT[64:128, i0 + off:i0 + CH],
                             start=True, stop=True, tile_position=(64, 0), skip_group_check=True)
            return pss

        SC = EXP_SCALE if DR_PROJ else 1.0
        n_up = min(2, NCH)          # chunks prepped upfront (short early chunks)
        emit_x8(0)
        emit_xtb(0)
        nc.gpsimd.memset(vaug[0][:], 1.0)
        nc.gpsimd.memset(vaug[1][:], 1.0)
        if FP8_AV:
            for b in range(2):
                nc.gpsimd.memset(vaug8[b][:], 0.0)
                nc.gpsimd.memset(
                    vaug8[b][:].rearrange("p (jb c) -> p jb c", c=128)[:, :, 64:65], 1.0)
        emit_chunk_prep(0, ws=(0, 1))
        for cc in range(1, n_up):
            emit_x8(cc)
        emit_chunk_prep(0, ws=(2,))
        for cc in range(1, n_up):
            emit_xtb(cc)
        emit_chunk_prep(0, ws=(3,))
        if n_up < NCH:
            emit_x8(n_up)
        for cc in range(1, n_up):
            emit_chunk_prep(cc)
        if n_up < NCH:
            emit_xtb(n_up)
        pending_b = None
        for c in range(NCH):
            pso = av_pool.tile([128 if FP8_AV else 65, 1024], f32, tag="av")
            njb = 4 * (c + 1)
            mid = max(4, njb // 3)
            pair_pend = None     # half-filled fp8 exp pair (FP8_AV)
            for jb in range(njb):
                # ---- S^T + exp first: keep ScalarE fed ----
                t = jb - 4 * c
                off = 128 * t if t > 0 else 0
                pss = emit_s(c, jb, off)
                use_fp8 = FP8_AV and t < 0
                if use_fp8:
                    if jb % 2 == 0:
                        pt2 = ptp.tile([128, 2048], fp8, tag="pt8", name="pt8")
                        nc.scalar.activation(r3(pt2)[:, 0, :], pss[:, :], Exp, scale=SC)
                        pair_pend = pt2
                    else:
                        pt2 = pair_pend
                        pair_pend = None
                        nc.scalar.activation(r3(pt2)[:, 1, :], pss[:, :], Exp, scale=SC)
                else:
                    pt = ptp.tile([128, 1024], bf16, tag="pt")
                    if off:
                        sub = lambda ap: ap.rearrange("p (h w) -> p h w", h=2)[:, :, off:]
                        nc.scalar.activation(sub(pt[:]), sub(pss[:]), Exp, scale=SC)
                    else:
                        nc.scalar.activation(pt[:], pss[:], Exp, scale=SC)
                # ---- injected PE work rides under the exp ----
                # q/k projections run TWO chunks ahead so the repack DMAs are
                # long done when the chunk starts; v prep one chunk ahead.
                if jb == 2:
                    if pending_b is not None:
                        emit_epilogue_b(*pending_b, dblks=(0, 1))
                    if n_up <= c + 2 < NCH and (c + 2) not in xts_pend:
                        emit_xt(c + 2)
                if jb == 3:
                    if pending_b is not None:
                        emit_epilogue_b(*pending_b, dblks=(2, 3))
                        pending_b = None
                    if c == 0 and n_up <= 2 < NCH:
                        emit_chunk_prep(2, ws=(0, 1))
                if jb == mid and n_up <= c + 1 < NCH:
                    emit_chunk_prep(c + 1, ws=(2,))
                if jb == mid + 1 and n_up <= c + 1 < NCH:
                    emit_chunk_prep(c + 1, ws=(3,))
                if jb == mid + 2 and c >= 1 and n_up <= c + 2 < NCH:
                    emit_chunk_prep(c + 2, ws=(0,))
                if jb == mid + 3 and c >= 1 and n_up <= c + 2 < NCH:
                    emit_chunk_prep(c + 2, ws=(1,))
                # ---- mask + A@V ----
                if use_fp8:
                    if jb % 2 == 1:      # pair complete: one DoubleRow A@V
                        pp = jb // 2
                        for b in range(2):
                            nc.tensor.matmul(
                                pso[0:128, 512 * b:512 * (b + 1)],
                                r3(vaug8[b][:, 256 * pp:256 * (pp + 1)], t=2),
                                r3(pt2)[:, :, 512 * b:512 * (b + 1)],
                                start=(jb == 1), stop=False, perf_mode=DR,
                                skip_group_check=True)
                    continue
                if t >= 0:
                    # only the 128-col diagonal square is partially masked
                    msub = pt[:].rearrange("p (h w) -> p h w", h=2)[:, :, off:off + 128]
                    nc.vector.tensor_tensor(
                        msub, msub,
                        mask_sb[:, 0:256].rearrange("p (h w) -> p h w", h=2), mult)
                first = (jb == 0)
                nc.tensor.matmul(pso[0:65, off:512], vaug[0][:, 65 * jb:65 * jb + 65],
                                 pt[:, off:512],
                                 start=first, stop=(jb == njb - 1), skip_group_check=True)
                nc.tensor.matmul(pso[0:65, 512 + off:1024], vaug[1][:, 65 * jb:65 * jb + 65],
                                 pt[:, 512 + off:1024],
                                 start=first, stop=(jb == njb - 1), skip_group_check=True)

            outTn = emit_epilogue_a(c, pso)
            if (c + 2) < NCH and (c + 2) not in xts_pend and n_up <= c + 2:
                emit_xt(c + 2)   # fallback if the jb==2 site did not fire
            pending_b = (c, outTn)
        if pending_b is not None:
            emit_epilogue_b(*pending_b)
    return nc


def make_host_constants(NB: int):
    """0/1 masks for the diagonal j-block square and the stacked identity."""
    jj = np.arange(JB)[:, None]
    ii = np.arange(JB)[None, :]
    m = (ii >= jj).astype(np.float32)                    # [128, 128] diagonal square
    masks = np.concatenate([m, m], axis=1)               # [128, 256]
    identup = np.concatenate([np.eye(DH, dtype=np.float32)] * 2, axis=0)  # [128, 64]
    return (masks.astype(ml_dtypes.bfloat16), identup.astype(ml_dtypes.bfloat16))


_CACHE = {}


def _get_compiled(NB: int):
    key = ("nc", NB)
    if key not in _CACHE:
        import concourse.bacc as bacc
        nc = bacc.Bacc("TRN2", debug=False, num_devices=N_CORES)
        build_attention_kernel(nc, NB)
        nc.compile()
        _CACHE[key] = nc
    return _CACHE[key]


def make_in_maps(x, Wq, Wkv, Wo, NB: int):
    bf = ml_dtypes.bfloat16
    f8 = ml_dtypes.float8_e4m3
    NB = x.shape[1]
    nb_total = x.shape[0] * NB
    xT = x.reshape(nb_total, DIM).T            # [512, B*NB], batch-major cols
    xT = xT.reshape(DIM, 2, NB // CH, CH).transpose(0, 2, 1, 3).reshape(DIM, nb_total)
    xT = np.ascontiguousarray(xT)              # chunk-paired: col = 1024c + 512b + i
    # fp8 copy in paired-d-tile layout: [128, pair(2), t(2), 2NB]
    x8 = xT.reshape(2, 2, 128, nb_total).transpose(2, 0, 1, 3).reshape(128, 4 * nb_total)
    x8 = np.ascontiguousarray(x8).astype(f8)
    xT_bf = xT.astype(bf)
    masks, identup = make_host_constants(NB)
    in_maps = []

    def wqk8pack(wq, wk):
        # [512, 64]x2 -> [128, 512] fp8: col = pair*256 + t*128 + (q|k index)
        wqk = np.concatenate([wq, wk], axis=1)           # [512, 128]
        return np.ascontiguousarray(
            wqk.reshape(2, 2, 128, 128).transpose(2, 0, 1, 3).reshape(128, 512)
        ).astype(f8)

    def wpack(w):        # [512, 64] -> SBUF layout [128, 256] (d-tile on free dim)
        return np.ascontiguousarray(
            w.reshape(4, 128, DH).transpose(1, 0, 2).reshape(128, 4 * DH)).astype(bf)

    for h in range(N_CORES):
        s = slice(DH * h, DH * (h + 1))
        im = {
            "xT": xT_bf,
            "wv": wpack(Wkv[:, DIM + DH * h:DIM + DH * (h + 1)]),
            "wo": np.ascontiguousarray(Wo[s, :]).astype(bf),
            "masks": masks,
            "identup": identup,
        }
        if DR_PROJ:
            im["x8"] = x8
            im["wqk8"] = wqk8pack(Wq[:, s] * 16.0, Wkv[:, DH * h:DH * (h + 1)] * 16.0)
        else:
            im["wqb"] = wpack(Wq[:, s] / 64.0)
            im["wkb"] = wpack(Wkv[:, DH * h:DH * (h + 1)])
        in_maps.append(im)
    return in_maps


def kernel(x, Wq, Wkv, Wo, bo, _run_kwargs=None):
    from concourse.bass_utils import run_bass_kernel_spmd
    x = np.asarray(x, np.float32)
    NB = x.shape[1]
    nc = _get_compiled(NB)
    in_maps = make_in_maps(np.asarray(x), np.asarray(Wq), np.asarray(Wkv), np.asarray(Wo), NB)
    res = run_bass_kernel_spmd(nc, in_maps, core_ids=list(range(N_CORES)),
                               **(_run_kwargs or {}))
    oT = np.zeros((DIM, x.shape[0] * NB), np.float32)
    for c in range(N_CORES):
        den = res.results[c]["den"].astype(np.float32).reshape(1, -1)
        oT += res.results[c]["oT"].astype(np.float32) / den
    # invert chunk-paired layout: col = 1024c + 512b + i  ->  [b, n, D]
    out = (oT.reshape(DIM, NB // CH, 2, CH).transpose(2, 1, 3, 0)
           .reshape(x.shape[0], NB, DIM).astype(np.float32) + np.asarray(bo, np.float32))
    if _run_kwargs is not None:
        _CACHE["last_results"] = res
    return out
